# revision 1
# baseline (speedup 1.0000x reference)
import sys

if '/opt/trn_rl_repo' not in sys.path:
    sys.path.insert(0, '/opt/trn_rl_repo')

import numpy as np
import concourse.bass as bass
from concourse import bacc
import concourse.mybir as mybir
from concourse.tile import TileContext
from concourse.bass_utils import run_bass_kernel_spmd

NCORES = 8
B, N, DM = 64, 256, 2048          # queries (B, N, DM)
NB, NP, DB = 8, 1024, 256         # blocks, prototypes/block, d_block
BN = B * N                        # 16384 total query rows
QPC = BN // NCORES                # 2048 query rows per core
NTILES = QPC // 128               # 16 n-tiles of 128 rows per core

_NC = None


def _build():
    """Per-core Bass program: 3-pass fp16 hi/lo matmul scores + K=2 bias,
    top-1 via DVE max/max_index, indirect-DMA codebook gather."""
    nc = bacc.Bacc()
    qhT = nc.declare_dram_parameter("qhT", [NTILES, 128, 16, 128], mybir.dt.float16, isOutput=False)
    qlT = nc.declare_dram_parameter("qlT", [NTILES, 128, 16, 128], mybir.dt.float16, isOutput=False)
    whT = nc.declare_dram_parameter("whT", [128, NB, 2, 2, 512], mybir.dt.float16, isOutput=False)
    wlT = nc.declare_dram_parameter("wlT", [128, NB, 2, 2, 512], mybir.dt.float16, isOutput=False)
    biasT = nc.declare_dram_parameter("biasT", [2, NB, 2, 512], mybir.dt.float16, isOutput=False)
    memd = nc.declare_dram_parameter("memd", [NB * NP, DB], mybir.dt.float32, isOutput=False)
    emb = nc.declare_dram_parameter("emb", [QPC, DM], mybir.dt.float32, isOutput=True)
    idx = nc.declare_dram_parameter("idx", [QPC, NB], mybir.dt.uint32, isOutput=True)
    smax = nc.declare_dram_parameter("smax", [QPC, NB], mybir.dt.float32, isOutput=True)

    with TileContext(nc) as tc:
        with tc.tile_pool(name="wsb", bufs=1) as wpool, \
             tc.tile_pool(name="qsb", bufs=2) as qpool, \
             tc.tile_pool(name="esb", bufs=2) as epool, \
             tc.tile_pool(name="msb", bufs=2) as mpool, \
             tc.tile_pool(name="psum", bufs=4, space="PSUM") as ppool:
            WH = wpool.tile([128, NB, 2, 2, 512], mybir.dt.float16)
            WL = wpool.tile([128, NB, 2, 2, 512], mybir.dt.float16)
            BS = wpool.tile([2, NB, 2, 512], mybir.dt.float16)
            nc.sync.dma_start(out=WH, in_=whT[:])
            nc.sync.dma_start(out=WL, in_=wlT[:])
            nc.sync.dma_start(out=BS, in_=biasT[:])
            ONES = wpool.tile([2, 128], mybir.dt.float16)
            nc.vector.memset(ONES, 1.0)

            for t in range(NTILES):
                QH = qpool.tile([128, 16, 128], mybir.dt.float16, name="QH")
                QL = qpool.tile([128, 16, 128], mybir.dt.float16, name="QL")
                nc.sync.dma_start(out=QH, in_=qhT[t])
                nc.sync.dma_start(out=QL, in_=qlT[t])
                OFFS = epool.tile([128, NB], mybir.dt.uint32, name="OFFS")
                SMX = epool.tile([128, NB], mybir.dt.float32, name="SMX")
                EMB = epool.tile([128, DM], mybir.dt.float32, name="EMB")
                for j in range(NB):
                    P = ppool.tile([128, 1024], mybir.dt.float32, name="P")
                    for h in range(2):
                        ph = P[:, h * 512:(h + 1) * 512]
                        nc.tensor.matmul(ph, ONES[:], BS[:, j, h, :], start=True, stop=False)
                        nc.tensor.matmul(ph, QH[:, 2 * j, :], WH[:, j, 0, h, :], start=False, stop=False)
                        nc.tensor.matmul(ph, QH[:, 2 * j, :], WL[:, j, 0, h, :], start=False, stop=False)
                        nc.tensor.matmul(ph, QH[:, 2 * j + 1, :], WH[:, j, 1, h, :], start=False, stop=False)
                        nc.tensor.matmul(ph, QH[:, 2 * j + 1, :], WL[:, j, 1, h, :], start=False, stop=False)
                        nc.tensor.matmul(ph, QL[:, 2 * j, :], WH[:, j, 0, h, :], start=False, stop=False)
                        nc.tensor.matmul(ph, QL[:, 2 * j + 1, :], WH[:, j, 1, h, :], start=False, stop=True)
                    MX = mpool.tile([128, 8], mybir.dt.float32, name="MX")
                    MI = mpool.tile([128, 8], mybir.dt.uint32, name="MI")
                    nc.vector.max(out=MX, in_=P)
                    nc.vector.max_index(out=MI, in_max=MX, in_values=P)
                    nc.vector.tensor_scalar_add(out=OFFS[:, j:j + 1], in0=MI[:, 0:1], scalar1=j * NP)
                    nc.vector.tensor_copy(out=SMX[:, j:j + 1], in_=MX[:, 0:1])
                for j in range(NB):
                    nc.gpsimd.indirect_dma_start(
                        out=EMB[:, j * DB:(j + 1) * DB], out_offset=None,
                        in_=memd[:],
                        in_offset=bass.IndirectOffsetOnAxis(ap=OFFS[:, j:j + 1], axis=0),
                    )
                nc.sync.dma_start(out=emb[t * 128:(t + 1) * 128, :], in_=EMB)
                nc.sync.dma_start(out=idx[t * 128:(t + 1) * 128, :], in_=OFFS)
                nc.sync.dma_start(out=smax[t * 128:(t + 1) * 128, :], in_=SMX)

    nc.finalize()
    return nc


def _get_nc():
    global _NC
    if _NC is None:
        _NC = _build()
    return _NC


def _prep_inputs(queries, mem):
    qr = np.ascontiguousarray(queries.reshape(BN, DM))
    qh = qr.astype(np.float16)
    ql = (qr - qh.astype(np.float32)).astype(np.float16)

    w = mem.reshape(NB, NP, DB)
    wh = w.astype(np.float16)
    wl = (w - wh.astype(np.float32)).astype(np.float16)
    # [pp, j, cc, h, nn] = w[j, h*512+nn, cc*128+pp]
    whT = np.ascontiguousarray(wh.reshape(NB, 2, 512, 2, 128).transpose(4, 0, 3, 1, 2))
    wlT = np.ascontiguousarray(wl.reshape(NB, 2, 512, 2, 128).transpose(4, 0, 3, 1, 2))

    k2n = (-0.5 * (w.astype(np.float64) ** 2).sum(-1)).astype(np.float32)  # (NB, NP)
    kh = k2n.astype(np.float16)
    kl = (k2n - kh.astype(np.float32)).astype(np.float16)
    biasT = np.ascontiguousarray(
        np.stack([kh, kl], 0).reshape(2, NB, 2, 512))

    in_maps = []
    for c in range(NCORES):
        sl = slice(c * QPC, (c + 1) * QPC)
        # [t, p, cdim, nn] = q[t*128+nn, cdim*128+p]
        qhT = np.ascontiguousarray(
            qh[sl].reshape(NTILES, 128, 16, 128).transpose(0, 3, 2, 1))
        qlT = np.ascontiguousarray(
            ql[sl].reshape(NTILES, 128, 16, 128).transpose(0, 3, 2, 1))
        in_maps.append({"qhT": qhT, "qlT": qlT, "whT": whT, "wlT": wlT,
                        "biasT": biasT, "memd": mem})
    return qr, in_maps


def _postprocess(qr, results):
    emb = np.concatenate([r["emb"] for r in results], 0).reshape(B, N, DM)
    idx = np.concatenate([r["idx"] for r in results], 0).astype(np.int32).reshape(B, N, NB)
    ssum = float(np.concatenate([r["smax"] for r in results], 0).astype(np.float64).sum())
    q2sum = 0.0
    for c in range(0, BN, 2048):
        ch = qr[c:c + 2048].astype(np.float64)
        q2sum += float((ch * ch).sum())
    closs = np.float32((q2sum - 2.0 * ssum) / (BN * DM))
    return emb, idx, np.float32(0.0), closs


def kernel(queries, mem):
    nc = _get_nc()
    qr, in_maps = _prep_inputs(np.asarray(queries), np.asarray(mem))
    res = run_bass_kernel_spmd(nc, in_maps, list(range(NCORES)))
    return _postprocess(qr, res.results)


# revision 4
# speedup vs baseline: 138383.5798x; 138383.5798x over previous
import sys

if '/opt/trn_rl_repo' not in sys.path:
    sys.path.insert(0, '/opt/trn_rl_repo')

import numpy as np
import concourse.bass as bass
from concourse import bacc
import concourse.mybir as mybir
from concourse.tile import TileContext

NCORES = 8
B, N, DM = 64, 256, 2048          # queries (B, N, DM)
NB, NP, DB = 8, 1024, 256         # blocks, prototypes/block, d_block
BN = B * N                        # 16384 total query rows
QPC = BN // NCORES                # 2048 query rows per core
NTILES = QPC // 128               # 16 n-tiles of 128 rows per core

_NC = None


def _build():
    """Per-core Bass program: 3-pass fp16 hi/lo matmul scores + K=2 bias,
    top-1 via DVE max/max_index, indirect-DMA codebook gather."""
    nc = bacc.Bacc()
    qhT = nc.declare_dram_parameter("qhT", [NTILES, 128, 16, 128], mybir.dt.float16, isOutput=False)
    qlT = nc.declare_dram_parameter("qlT", [NTILES, 128, 16, 128], mybir.dt.float16, isOutput=False)
    whT = nc.declare_dram_parameter("whT", [128, NB, 2, 2, 512], mybir.dt.float16, isOutput=False)
    wlT = nc.declare_dram_parameter("wlT", [128, NB, 2, 2, 512], mybir.dt.float16, isOutput=False)
    biasT = nc.declare_dram_parameter("biasT", [2, NB, 2, 512], mybir.dt.float16, isOutput=False)
    memd = nc.declare_dram_parameter("memd", [NB * NP, DB], mybir.dt.float32, isOutput=False)
    emb = nc.declare_dram_parameter("emb", [QPC, DM], mybir.dt.float32, isOutput=True)
    idx = nc.declare_dram_parameter("idx", [QPC, NB], mybir.dt.uint32, isOutput=True)
    smax = nc.declare_dram_parameter("smax", [QPC, NB], mybir.dt.float32, isOutput=True)

    with TileContext(nc) as tc:
        with tc.tile_pool(name="wsb", bufs=1) as wpool, \
             tc.tile_pool(name="qsb", bufs=2) as qpool, \
             tc.tile_pool(name="esb", bufs=2) as epool, \
             tc.tile_pool(name="msb", bufs=2) as mpool, \
             tc.tile_pool(name="psum", bufs=4, space="PSUM") as ppool:
            WH = wpool.tile([128, NB, 2, 2, 512], mybir.dt.float16)
            WL = wpool.tile([128, NB, 2, 2, 512], mybir.dt.float16)
            BS = wpool.tile([2, NB, 2, 512], mybir.dt.float16)
            nc.sync.dma_start(out=WH, in_=whT[:])
            nc.sync.dma_start(out=WL, in_=wlT[:])
            nc.sync.dma_start(out=BS, in_=biasT[:])
            ONES = wpool.tile([2, 128], mybir.dt.float16)
            nc.vector.memset(ONES, 1.0)

            for t in range(NTILES):
                QH = qpool.tile([128, 16, 128], mybir.dt.float16, name="QH")
                QL = qpool.tile([128, 16, 128], mybir.dt.float16, name="QL")
                nc.sync.dma_start(out=QH, in_=qhT[t])
                nc.sync.dma_start(out=QL, in_=qlT[t])
                OFFS = epool.tile([128, NB], mybir.dt.uint32, name="OFFS")
                SMX = epool.tile([128, NB], mybir.dt.float32, name="SMX")
                EMB = epool.tile([128, DM], mybir.dt.float32, name="EMB")
                for j in range(NB):
                    P = ppool.tile([128, 1024], mybir.dt.float32, name="P")
                    for h in range(2):
                        ph = P[:, h * 512:(h + 1) * 512]
                        nc.tensor.matmul(ph, ONES[:], BS[:, j, h, :], start=True, stop=False)
                        nc.tensor.matmul(ph, QH[:, 2 * j, :], WH[:, j, 0, h, :], start=False, stop=False)
                        nc.tensor.matmul(ph, QH[:, 2 * j, :], WL[:, j, 0, h, :], start=False, stop=False)
                        nc.tensor.matmul(ph, QH[:, 2 * j + 1, :], WH[:, j, 1, h, :], start=False, stop=False)
                        nc.tensor.matmul(ph, QH[:, 2 * j + 1, :], WL[:, j, 1, h, :], start=False, stop=False)
                        nc.tensor.matmul(ph, QL[:, 2 * j, :], WH[:, j, 0, h, :], start=False, stop=False)
                        nc.tensor.matmul(ph, QL[:, 2 * j + 1, :], WH[:, j, 1, h, :], start=False, stop=True)
                    MX = mpool.tile([128, 8], mybir.dt.float32, name="MX")
                    MI = mpool.tile([128, 8], mybir.dt.uint32, name="MI")
                    nc.vector.max(out=MX, in_=P)
                    nc.vector.max_index(out=MI, in_max=MX, in_values=P)
                    nc.vector.tensor_scalar_add(out=OFFS[:, j:j + 1], in0=MI[:, 0:1], scalar1=j * NP)
                    nc.vector.tensor_copy(out=SMX[:, j:j + 1], in_=MX[:, 0:1])
                for j in range(NB):
                    nc.gpsimd.indirect_dma_start(
                        out=EMB[:, j * DB:(j + 1) * DB], out_offset=None,
                        in_=memd[:],
                        in_offset=bass.IndirectOffsetOnAxis(ap=OFFS[:, j:j + 1], axis=0),
                    )
                nc.sync.dma_start(out=emb[t * 128:(t + 1) * 128, :], in_=EMB)
                nc.sync.dma_start(out=idx[t * 128:(t + 1) * 128, :], in_=OFFS)
                nc.sync.dma_start(out=smax[t * 128:(t + 1) * 128, :], in_=SMX)

    nc.finalize()
    return nc


class _Runner:
    """Caches the jitted shard_map executable so repeat calls skip XLA/NEFF
    compilation (bass2jax.run_bass_via_pjrt re-jits per call)."""

    def __init__(self):
        self.nc = _build()
        import jax
        from jax.sharding import Mesh, PartitionSpec
        from jax.experimental.shard_map import shard_map
        from concourse import bass2jax
        bass2jax.install_neuronx_cc_hook()
        nc = self.nc
        partition_name = nc.partition_id_tensor.name if nc.partition_id_tensor else None
        in_names, out_names, out_avals, zero_tmpl = [], [], [], []
        for alloc in nc.m.functions[0].allocations:
            if not isinstance(alloc, mybir.MemoryLocationSet):
                continue
            name = alloc.memorylocations[0].name
            if alloc.kind == "ExternalInput":
                if name != partition_name:
                    in_names.append(name)
            elif alloc.kind == "ExternalOutput":
                shape = tuple(alloc.tensor_shape)
                dtype = mybir.dt.np(alloc.dtype)
                out_names.append(name)
                out_avals.append(jax.core.ShapedArray(shape, dtype))
                zero_tmpl.append((shape, dtype))
        self.param_names = list(in_names)
        self.out_names = out_names
        self.out_avals = out_avals
        self.zero_tmpl = zero_tmpl
        n_params = len(in_names)
        n_outs = len(out_names)
        all_in_names = in_names + out_names + ([partition_name] if partition_name else [])

        def _body(*args):
            operands = list(args)
            if partition_name is not None:
                operands.append(bass2jax.partition_id_tensor())
            outs = bass2jax._bass_exec_p.bind(
                *operands,
                out_avals=tuple(out_avals),
                in_names=tuple(all_in_names),
                out_names=tuple(out_names),
                lowering_input_output_aliases=(),
                sim_require_finite=True,
                sim_require_nnan=True,
                nc=nc,
            )
            return tuple(outs)

        devices = jax.devices()[:NCORES]
        self.mesh = Mesh(np.asarray(devices), ("core",))
        self.pspec = PartitionSpec("core")
        in_specs = (self.pspec,) * (n_params + n_outs)
        out_specs = (self.pspec,) * n_outs
        self.donate = tuple(range(n_params, n_params + n_outs))
        self.sharded = jax.jit(
            shard_map(_body, mesh=self.mesh, in_specs=in_specs,
                      out_specs=out_specs, check_rep=False),
            donate_argnums=self.donate, keep_unused=True,
        )

    def concat_inputs(self, in_maps):
        return [np.concatenate([m[name] for m in in_maps], axis=0)
                for name in self.param_names]

    def make_zeros(self):
        return [np.zeros((NCORES * s[0], *s[1:]), d) for s, d in self.zero_tmpl]

    def run(self, in_maps):
        out_arrs = self.sharded(*self.concat_inputs(in_maps), *self.make_zeros())
        return [
            {name: np.asarray(out_arrs[i]).reshape(NCORES, *self.out_avals[i].shape)[c]
             for i, name in enumerate(self.out_names)}
            for c in range(NCORES)
        ]


def _get_runner():
    global _NC
    if _NC is None:
        _NC = _Runner()
    return _NC


def _prep_inputs(queries, mem):
    qr = np.ascontiguousarray(queries.reshape(BN, DM))
    qh = qr.astype(np.float16)
    ql = (qr - qh.astype(np.float32)).astype(np.float16)

    w = mem.reshape(NB, NP, DB)
    wh = w.astype(np.float16)
    wl = (w - wh.astype(np.float32)).astype(np.float16)
    # [pp, j, cc, h, nn] = w[j, h*512+nn, cc*128+pp]
    whT = np.ascontiguousarray(wh.reshape(NB, 2, 512, 2, 128).transpose(4, 0, 3, 1, 2))
    wlT = np.ascontiguousarray(wl.reshape(NB, 2, 512, 2, 128).transpose(4, 0, 3, 1, 2))

    k2n = (-0.5 * (w.astype(np.float64) ** 2).sum(-1)).astype(np.float32)  # (NB, NP)
    kh = k2n.astype(np.float16)
    kl = (k2n - kh.astype(np.float32)).astype(np.float16)
    biasT = np.ascontiguousarray(
        np.stack([kh, kl], 0).reshape(2, NB, 2, 512))

    in_maps = []
    for c in range(NCORES):
        sl = slice(c * QPC, (c + 1) * QPC)
        # [t, p, cdim, nn] = q[t*128+nn, cdim*128+p]
        qhT = np.ascontiguousarray(
            qh[sl].reshape(NTILES, 128, 16, 128).transpose(0, 3, 2, 1))
        qlT = np.ascontiguousarray(
            ql[sl].reshape(NTILES, 128, 16, 128).transpose(0, 3, 2, 1))
        in_maps.append({"qhT": qhT, "qlT": qlT, "whT": whT, "wlT": wlT,
                        "biasT": biasT, "memd": mem})
    return qr, in_maps


def _postprocess(qr, results):
    emb = np.concatenate([r["emb"] for r in results], 0).reshape(B, N, DM)
    idx = np.concatenate([r["idx"] for r in results], 0).astype(np.int32).reshape(B, N, NB)
    ssum = float(np.concatenate([r["smax"] for r in results], 0).astype(np.float64).sum())
    q2sum = 0.0
    for c in range(0, BN, 2048):
        ch = qr[c:c + 2048].astype(np.float64)
        q2sum += float((ch * ch).sum())
    closs = np.float32((q2sum - 2.0 * ssum) / (BN * DM))
    return emb, idx, np.float32(0.0), closs


def kernel(queries, mem):
    runner = _get_runner()
    qr, in_maps = _prep_inputs(np.asarray(queries), np.asarray(mem))
    return _postprocess(qr, runner.run(in_maps))
